# revision 7
# baseline (speedup 1.0000x reference)
"""Causal self-attention Trainium2 Bass kernel — static-instruction-minimal.

Empirically this environment charges ~65us per STATIC instruction in the
NEFF (load/dispatch dominated), while executed instructions, matmul width,
and DMA descriptor counts are ~free.  The kernel therefore wraps all work
in dynamic For_i loops so the static program is ~400 instructions instead
of the ~2100 of a fully unrolled kernel.

Key devices to keep every loop-varying operand off the matmul stationary
port (walrus forbids register offsets in ldweights):
  - stationary tiles are staged into fixed scratch slots by DMA/DVE copies
    with dynamic source offsets;
  - PSUM accumulation groups are opened/closed by zero-stationary matmuls
    so start/stop flags stay static inside loops;
  - q/k and y are stored in head-major duplicated layouts ([64, slot, t])
    so all head/chunk addressing is affine in loop registers;
  - scores are computed full-width (no causal windowing — flops are free)
    and masked post-exp with a sliced static triangular mask;
  - the projection is computed transposed (out^T) so weights are the
    stationary; the host transposes back.

Sharding: 8 cores = (batch b in 0..3) x (head-group g in 0..1, 8 heads).
Core (b, g) computes a partial projection output for batch b over its 512
model dims; the host sums the two partials per batch (b_proj folded into
the g==0 core only).  Matmul inputs are shipped as bf16 (tolerance 2e-2).
"""

import numpy as np
import ml_dtypes

import concourse.bacc as bacc
import concourse.mybir as mybir
import concourse.tile as tile
from concourse.bass import ds
from concourse.bass_utils import run_bass_kernel_spmd

F32 = mybir.dt.float32
BF16 = mybir.dt.bfloat16
AF = mybir.ActivationFunctionType
BF = ml_dtypes.bfloat16

B, T, D, H = 4, 2048, 1024, 16
HD = 64              # head dim
HPC = 8              # heads per core
DC = HPC * HD        # 512 model dims per core
SCALE = 1.0 / np.sqrt(HD)

_NC_CACHE = {}


def build_nc(t=T, reps=1, with_bias=False):
    """Build the single-core SPMD program. t must be T for real runs.
    with_bias adds the ones x bias-row matmuls (spec fills biases with
    zeros, so the default program omits them)."""
    nt = t // 128
    KC = D // 128          # 8 contraction chunks over model dim

    nc = bacc.Bacc("TRN2", target_bir_lowering=False, debug=False)

    xT_d = nc.dram_tensor("xT", [D, t], BF16, kind="ExternalInput")
    # weight matrices carry the bias as an extra contraction row (vs ones)
    wqkv_d = nc.dram_tensor("wqkv", [D + 1, 3 * DC], BF16, kind="ExternalInput")
    wp_d = nc.dram_tensor("wp", [DC + 1, D], BF16, kind="ExternalInput")
    outT_d = nc.dram_tensor("outT", [D, t], F32, kind="ExternalOutput")

    NM = 3 * DC // 128     # 12 output chunks of 128 feats (q, k, v)

    with tile.TileContext(nc) as tc:
      for _rep in range(reps):
        with tc.tile_pool(name="ptop", bufs=1) as ptop:
          # normalized y^T, head-major: slot h rows 0:64 = head h dims
          ydup = ptop.tile([64, HPC, t], BF16)

          with tc.tile_pool(name="pab", bufs=1) as pab:
            # q/k/v head-major dup: slots 0-7 q^T, 8-15 k^T, 16-23 v^T
            qkvdup = pab.tile([64, 3 * HPC, t], BF16)
            # v natural, per head contiguous: vnat[p, h, tt, d] = v[tt*128+p, h, d]
            vnat = pab.tile([128, HPC, nt, HD], BF16, name="vnat", tag="vnat")
            # causal mask: Tm[p, u] = 1 iff u - t - p >= 0; slice
            # Tm[:, t - 128*kc :][:, :t] masks keys chunk kc vs all queries
            Tm = pab.tile([128, 2 * t], BF16, name="Tm", tag="Tm")
            onesr = pab.tile([1, t], BF16, name="onesr", tag="onesr")

            if with_bias:
                nc.gpsimd.memset(onesr[:], 1.0)
            nc.gpsimd.memset(Tm[:], 1.0)
            nc.gpsimd.affine_select(
                out=Tm[:], in_=Tm[:], compare_op=mybir.AluOpType.is_ge,
                fill=0.0, base=-t, pattern=[[1, 2 * t]], channel_multiplier=-1)

            # ---------------- Phase A: qkv ----------------
            with tc.tile_pool(name="pa", bufs=1) as pa, \
                 tc.tile_pool(name="pa_ps", bufs=1, space="PSUM") as paps:
                xTb = pa.tile([128, KC, t], BF16)
                ws = pa.tile([128, KC, 128], BF16, name="ws", tag="ws")
                wsb = pa.tile([1, 128], BF16, name="wsb", tag="wsb")
                nc.sync.dma_start(xTb[:], xT_d.rearrange("(k p) t -> p k t", p=128))

                psA = paps.tile([128, t], F32, name="psA", tag="psA")
                with tc.For_i(0, NM) as m:
                    nc.sync.dma_start(
                        ws[:], wqkv_d[0:D, ds(m * 128, 128)]
                        .rearrange("(k p) c -> p k c", p=128))
                    if with_bias:
                        nc.sync.dma_start(
                            wsb[:], wqkv_d[D:D + 1, ds(m * 128, 128)])
                    for k in range(KC):
                        for w in range(t // 512):
                            nc.tensor.matmul(
                                psA[:, w * 512:(w + 1) * 512],
                                ws[:, k, :], xTb[:, k, w * 512:(w + 1) * 512],
                                start=(k == 0),
                                stop=(not with_bias and k == KC - 1))
                    if with_bias:
                        for w in range(t // 512):
                            nc.tensor.matmul(
                                psA[:, w * 512:(w + 1) * 512],
                                wsb[:], onesr[:, w * 512:(w + 1) * 512],
                                start=False, stop=True)
                    nc.vector.tensor_copy(
                        qkvdup[:, ds(m * 2, 1), :], psA[0:64, :].unsqueeze(1))
                    nc.vector.tensor_copy(
                        qkvdup[:, ds(m * 2 + 1, 1), :], psA[64:128, :].unsqueeze(1))

                # v^T slots -> v natural via DMA transpose (contiguous dst;
                # HW writes dst[p, tt, d] = src[d, tt*128 + p])
                for h in range(HPC):
                    nc.sync.dma_start(
                        vnat[:, h, :, :], qkvdup[:, 2 * HPC + h, :],
                        transpose=True)

            # ---------------- Phase B: attention ----------------
            with tc.tile_pool(name="pb", bufs=1) as pb, \
                 tc.tile_pool(name="pb_ps", bufs=1, space="PSUM") as pbps:
                khs = pb.tile([64, 128], BF16, name="khs", tag="khs")
                # vhs col HD stays 1.0 (softmax denominator ones column)
                vhs = pb.tile([128, HD + 1], BF16, name="vhs", tag="vhs")
                nc.gpsimd.memset(vhs[:], 1.0)
                esb = pb.tile([128, t], BF16, name="esb", tag="esb")
                rec = pb.tile([1, t], F32, name="rec", tag="rec")
                rb = pb.tile([64, t], F32, name="rb", tag="rb")
                sp = pbps.tile([128, t], F32, name="sp", tag="sp")
                yacc = pbps.tile([HD + 1, t], F32, name="yacc", tag="yacc")
                with tc.For_i(0, HPC) as h:
                    nc.vector.memset(yacc[:], 0.0)
                    with tc.For_i(0, nt) as kc:
                        nc.vector.tensor_copy(
                            khs[:].unsqueeze(1),
                            qkvdup[:, ds(HPC + h, 1), ds(kc * 128, 128)])
                        for w in range(t // 512):
                            nc.tensor.matmul(
                                sp[:, w * 512:(w + 1) * 512],
                                khs[:],
                                qkvdup[:, ds(h, 1), w * 512:(w + 1) * 512],
                                start=True, stop=True)
                        nc.scalar.activation(esb[:], sp[:], AF.Exp,
                                             scale=float(SCALE))
                        nc.vector.tensor_mul(esb[:], esb[:],
                                             Tm[:, ds(t - kc * 128, t)])
                        nc.vector.tensor_copy(
                            vhs[:, 0:HD].unsqueeze(1),
                            vnat[:, ds(h, 1), ds(kc, 1), :]
                            .rearrange("p a b e -> p (a b) e"))
                        for w in range(t // 512):
                            nc.tensor.matmul(
                                yacc[:, w * 512:(w + 1) * 512],
                                vhs[:], esb[:, w * 512:(w + 1) * 512],
                                start=False, stop=False,
                                skip_group_check=True)
                    nc.vector.reciprocal(rec[:], yacc[HD:HD + 1, :])
                    nc.gpsimd.partition_broadcast(rb[:], rec[:])
                    nc.vector.tensor_mul(
                        ydup[:, ds(h, 1), :],
                        yacc[0:HD, :].unsqueeze(1), rb[:].unsqueeze(1))

          # ---------------- Phase C: projection (transposed) ----------------
          with tc.tile_pool(name="pc", bufs=1) as pc, \
               tc.tile_pool(name="pc_ps", bufs=1, space="PSUM") as pcps:
            osb = pc.tile([128, D // 128, t], F32, name="osb", tag="osb")
            # repack head-major y (K=64 slots) into dense [128, 4, t] so the
            # projection contracts in 4 K=128 steps instead of 8 K=64
            ypk = pc.tile([128, DC // 128, t], BF16, name="ypk", tag="ypk")
            for f in range(DC // 128):
                nc.sync.dma_start(ypk[0:64, f, :], ydup[:, 2 * f, :])
                nc.sync.dma_start(ypk[64:128, f, :], ydup[:, 2 * f + 1, :])
            wps = pc.tile([128, DC // 128, 128], BF16, name="wps", tag="wps")
            wpsb = pc.tile([1, 128], BF16, name="wpsb", tag="wpsb")
            onesc = pc.tile([1, t], BF16, name="onesc", tag="onesc")
            if with_bias:
                nc.gpsimd.memset(onesc[:], 1.0)
            pso = pcps.tile([128, t], F32, name="pso", tag="pso")
            with tc.For_i(0, D // 128) as oc:
                nc.sync.dma_start(
                    wps[:], wp_d[0:DC, ds(oc * 128, 128)]
                    .rearrange("(f p) c -> p f c", p=128))
                if with_bias:
                    nc.sync.dma_start(
                        wpsb[:], wp_d[DC:DC + 1, ds(oc * 128, 128)])
                for f in range(DC // 128):
                    for w in range(t // 512):
                        nc.tensor.matmul(
                            pso[:, w * 512:(w + 1) * 512],
                            wps[:, f, :], ypk[:, f, w * 512:(w + 1) * 512],
                            start=(f == 0),
                            stop=(not with_bias and f == DC // 128 - 1))
                if with_bias:
                    for w in range(t // 512):
                        nc.tensor.matmul(
                            pso[:, w * 512:(w + 1) * 512],
                            wpsb[:], onesc[:, w * 512:(w + 1) * 512],
                            start=False, stop=True)
                nc.vector.tensor_copy(
                    osb[:, ds(oc, 1), :], pso[:].unsqueeze(1))
            nc.sync.dma_start(outT_d.rearrange("(c p) t -> p c t", p=128), osb[:])

    nc.finalize()
    return nc


def make_in_maps(x, w_attn, b_attn, w_proj, b_proj):
    x = np.asarray(x, dtype=np.float32)
    w_attn = np.asarray(w_attn, dtype=np.float32)
    b_attn = np.asarray(b_attn, dtype=np.float32)
    w_proj = np.asarray(w_proj, dtype=np.float32)
    b_proj = np.asarray(b_proj, dtype=np.float32)
    in_maps = []
    for c in range(8):
        b, g = c // 2, c % 2
        sl = slice(DC * g, DC * (g + 1))
        wqkv = np.concatenate(
            [w_attn[:, 0 * D:1 * D][:, sl],
             w_attn[:, 1 * D:2 * D][:, sl],
             w_attn[:, 2 * D:3 * D][:, sl]], axis=1)
        bqkv = np.concatenate(
            [b_attn[0 * D:1 * D][sl], b_attn[1 * D:2 * D][sl],
             b_attn[2 * D:3 * D][sl]])[None, :]
        bp = (b_proj if g == 0 else np.zeros_like(b_proj))[None, :]
        in_maps.append({
            "xT": np.ascontiguousarray(x[b].T).astype(BF),
            "wqkv": np.ascontiguousarray(
                np.concatenate([wqkv, bqkv], axis=0)).astype(BF),
            "wp": np.ascontiguousarray(
                np.concatenate([w_proj[sl, :], bp], axis=0)).astype(BF),
        })
    return in_maps


def kernel(x, w_attn, b_attn, w_proj, b_proj, _trace=False, _trace_kwargs=None):
    with_bias = bool(np.any(np.asarray(b_attn)) or np.any(np.asarray(b_proj)))
    key = ("nc", with_bias)
    if key not in _NC_CACHE:
        _NC_CACHE[key] = build_nc(with_bias=with_bias)
    nc = _NC_CACHE[key]
    in_maps = make_in_maps(x, w_attn, b_attn, w_proj, b_proj)
    kw = {}
    if _trace:
        kw["trace"] = True
        if _trace_kwargs:
            kw.update(_trace_kwargs)
    res = run_bass_kernel_spmd(nc, in_maps, core_ids=list(range(8)), **kw)
    outs = [res.results[c]["outT"] for c in range(8)]
    out = np.empty((B, T, D), dtype=np.float32)
    for b in range(B):
        np.add(outs[2 * b].T, outs[2 * b + 1].T, out=out[b])
    kernel._last_results = res
    return out


if __name__ == "__main__":
    nc = build_nc()
    fn = nc.m.functions[0]
    n = sum(len(blk.instructions) for blk in fn.blocks)
    print(f"built ok, static instructions: {n}")
